# revision 6
# baseline (speedup 1.0000x reference)
"""CapsuleLayer dynamic-routing kernel for 8 Trainium2 NeuronCores.

Sharding: in_channels (ic=1152) split 8 ways (144 per core). Per routing
iteration each core computes its partial s_j over its c-slice; one AllReduce
per iteration sums s-partials (and the softmax denominator, folded into the
same buffer). u_hat is never materialized: both routing contractions are
expressed through the rank structure u_hat = W @ x.

Layouts (per core), with cl = local channel idx (144), i = in_unit (8),
flat k = cl*8 + i (KL = 1152 rows = 9 tiles of 128):
  xT [KL, 256]  : xT[k, b] = x[b, i, c]        (lhsT of the s-matmul)
  xF [256, KL]  : xF[b, k] = x[b, i, c]        (lhsT of the M-matmul)
  Wp [KL, 160]  : Wp[k, d*16+u] = W0[c, d, u, i]
  b_ij state    : b_sb[p, t*10+d] = b[16t + p//8, d]  (replicated over i = p%8)
"""

import sys

if "/opt/trn_rl_repo" not in sys.path:
    sys.path.insert(0, "/opt/trn_rl_repo")

import numpy as np

N_CORES = 8
B, IU, IC, D, U = 256, 8, 1152, 10, 16
CL = IC // N_CORES          # 144 channels per core
KL = CL * IU                # 1152 flat (cl, i) rows per core
NT = KL // 128              # 9 partition tiles
DU = D * U                  # 160
H = B // 128                # 2 batch chunks
NUM_ITERATIONS = 3

_CACHE = {}


def build_program(repeats=1):
    import concourse.mybir as mybir
    import concourse.tile as tile
    from concourse import bacc

    f32 = mybir.dt.float32
    ALU = mybir.AluOpType
    ACTF = mybir.ActivationFunctionType
    AX = mybir.AxisListType

    nc = bacc.Bacc(
        "TRN2",
        target_bir_lowering=False,
        debug=False,
        enable_asserts=False,
        num_devices=N_CORES,
    )

    xT = nc.dram_tensor("xT", [KL, B], f32, kind="ExternalInput")
    xF = nc.dram_tensor("xF", [B, KL], f32, kind="ExternalInput")
    Wp = nc.dram_tensor("Wp", [KL, DU], f32, kind="ExternalInput")
    sel = nc.dram_tensor("sel", [128, 128], f32, kind="ExternalInput")
    out = nc.dram_tensor("out", [B, DU], f32, kind="ExternalOutput")

    with tile.TileContext(nc) as tc:
        with (
            tc.tile_pool(name="big", bufs=1) as bigp,
            tc.tile_pool(name="small", bufs=1) as smp,
            tc.tile_pool(name="ps_s", bufs=2, space="PSUM") as ps_s,
            tc.tile_pool(name="ps_m", bufs=3, space="PSUM") as ps_m,
            tc.tile_pool(name="ps_x", bufs=1, space="PSUM") as ps_x,
            tc.tile_pool(name="dram", bufs=2, space="DRAM") as dpool,
        ):
            xT_sb = bigp.tile([128, NT * B], f32, tag="xT")
            xF_sb = bigp.tile([128, H * KL], f32, tag="xF")
            Wp_sb = bigp.tile([128, NT * DU], f32, tag="Wp")
            V_sb = bigp.tile([128, NT * DU], f32, tag="V")
            A_sb = bigp.tile([128, NT * DU], f32, tag="A")
            sel_sb = smp.tile([128, 128], f32, tag="sel")
            ones_sb = smp.tile([128, 1], f32, tag="ones")
            onesr_sb = smp.tile([1, 128], f32, tag="onesr")
            b_sb = smp.tile([128, NT * D], f32, tag="b")
            cexp_sb = smp.tile([128, NT * D], f32, tag="cexp")
            R_sb = smp.tile([128, NT * D], f32, tag="R")
            s_sb = smp.tile([128, H * DU], f32, tag="s")
            spart_sb = smp.tile([128, H * DU], f32, tag="spart")
            sn_sb = smp.tile([128, H * DU], f32, tag="sn")
            sq_sb = smp.tile([128, H * DU], f32, tag="sq")
            v_sb = smp.tile([128, H * DU], f32, tag="v")
            msq_sb = smp.tile([128, H * U], f32, tag="msq")
            sqm_sb = smp.tile([128, H * U], f32, tag="sqm")
            den_sb = smp.tile([128, H * U], f32, tag="den")
            rec_sb = smp.tile([128, H * U], f32, tag="rec")
            g_sb = smp.tile([128, H * U], f32, tag="g")
            zrow_sb = smp.tile([1, DU], f32, tag="zrow")
            ztmp_sb = smp.tile([1, D], f32, tag="ztmp")
            zinv_sb = smp.tile([1, D], f32, tag="zinv")

            def emit_pass():
                # ---- preload ----
                for t in range(NT):
                    nc.sync.dma_start(
                        out=xT_sb[:, t * B : (t + 1) * B],
                        in_=xT[t * 128 : (t + 1) * 128, :],
                    )
                    nc.sync.dma_start(
                        out=Wp_sb[:, t * DU : (t + 1) * DU],
                        in_=Wp[t * 128 : (t + 1) * 128, :],
                    )
                for h in range(H):
                    nc.sync.dma_start(
                        out=xF_sb[:, h * KL : (h + 1) * KL],
                        in_=xF[h * 128 : (h + 1) * 128, :],
                    )
                nc.sync.dma_start(out=sel_sb[:], in_=sel[:])
                nc.gpsimd.memset(ones_sb[:], 1.0)
                nc.gpsimd.memset(onesr_sb[:], 1.0)
                nc.gpsimd.memset(b_sb[:], 0.0)
                nc.gpsimd.memset(zrow_sb[:], 0.0)
                # iteration 0: c = exp(0) = 1 everywhere; local Z8 = 8*144
                nc.gpsimd.memset(zrow_sb[0:1, 0:D], float(IC))

                for it in range(NUM_ITERATIONS):
                    if it == 0:
                        Vcur = Wp_sb
                    else:
                        # c-scale: cexp = exp(b); Z8_local[d] = sum_{p,t} cexp
                        nc.scalar.activation(cexp_sb[:], b_sb[:], ACTF.Exp)
                        zsum_ps = ps_x.tile([1, NT * D], f32, tag="zsum")
                        nc.tensor.matmul(
                            zsum_ps[:], ones_sb[:], cexp_sb[:], start=True, stop=True
                        )
                        nc.vector.tensor_reduce(
                            out=zrow_sb[0:1, 0:D],
                            in_=zsum_ps[0:1, :].rearrange("p (t d) -> p d t", d=D),
                            axis=AX.X,
                            op=ALU.add,
                        )
                        nc.vector.tensor_mul(
                            V_sb[:].rearrange("p (t d u) -> p t d u", d=D, u=U),
                            Wp_sb[:].rearrange("p (t d u) -> p t d u", d=D, u=U),
                            cexp_sb[:]
                            .rearrange("p (t d) -> p t d", d=D)[:, :, :, None]
                            .broadcast_to([128, NT, D, U]),
                        )
                        Vcur = V_sb

                    # s-matmul: s_unnorm[b, (d,u)] = sum_k xT[k, b] * V[k, (d,u)]
                    s_tiles = []
                    for h in range(H):
                        sp = ps_s.tile([128, DU], f32, tag="s")
                        for t in range(NT):
                            nc.tensor.matmul(
                                sp[:],
                                xT_sb[:, t * B + h * 128 : t * B + h * 128 + 128],
                                Vcur[:, t * DU : (t + 1) * DU],
                                start=(t == 0),
                                stop=(t == NT - 1),
                            )
                        s_tiles.append(sp)

                    # AllReduce of [s_partial (256 rows) ; Z8 row]
                    cc_in = dpool.tile([B + 1, DU], f32, tag="ccin")
                    cc_out = dpool.tile([B + 1, DU], f32, tag="ccout")
                    for h in range(H):
                        nc.scalar.copy(
                            spart_sb[:, h * DU : (h + 1) * DU], s_tiles[h][:]
                        )
                        nc.sync.dma_start(
                            out=cc_in[h * 128 : (h + 1) * 128, :],
                            in_=spart_sb[:, h * DU : (h + 1) * DU],
                        )
                    nc.sync.dma_start(out=cc_in[B : B + 1, :], in_=zrow_sb[:])
                    nc.gpsimd.collective_compute(
                        "AllReduce",
                        ALU.add,
                        replica_groups=[list(range(N_CORES))],
                        ins=[cc_in.opt()],
                        outs=[cc_out.opt()],
                    )
                    for h in range(H):
                        nc.sync.dma_start(
                            out=s_sb[:, h * DU : (h + 1) * DU],
                            in_=cc_out[h * 128 : (h + 1) * 128, :],
                        )
                    nc.sync.dma_start(out=zrow_sb[:], in_=cc_out[B : B + 1, :])

                    # zinv[d] = 1 / (Z8_AR[d] / 8); broadcast across partitions
                    nc.scalar.mul(ztmp_sb[:], zrow_sb[0:1, 0:D], 1.0 / N_CORES)
                    nc.vector.reciprocal(zinv_sb[:], ztmp_sb[:])
                    zb_ps = ps_x.tile([128, D], f32, tag="zb")
                    nc.tensor.matmul(
                        zb_ps[:], onesr_sb[:], zinv_sb[:], start=True, stop=True
                    )

                    # squash (norm over d per (b, u), faithful to reference)
                    for h in range(H):
                        ssl = slice(h * DU, (h + 1) * DU)
                        nc.vector.tensor_mul(
                            sn_sb[:, ssl].rearrange("p (d u) -> p d u", u=U),
                            s_sb[:, ssl].rearrange("p (d u) -> p d u", u=U),
                            zb_ps[:, :, None].broadcast_to([128, D, U]),
                        )
                        nc.vector.tensor_mul(
                            sq_sb[:, ssl], sn_sb[:, ssl], sn_sb[:, ssl]
                        )
                        nc.vector.tensor_reduce(
                            out=msq_sb[:, h * U : (h + 1) * U],
                            in_=sq_sb[:, ssl].rearrange("p (d u) -> p u d", u=U),
                            axis=AX.X,
                            op=ALU.add,
                        )
                    nc.scalar.sqrt(sqm_sb[:], msq_sb[:])
                    nc.vector.scalar_tensor_tensor(
                        out=den_sb[:],
                        in0=msq_sb[:],
                        scalar=1.0,
                        in1=sqm_sb[:],
                        op0=ALU.add,
                        op1=ALU.mult,
                    )
                    nc.vector.reciprocal(rec_sb[:], den_sb[:])
                    nc.vector.tensor_mul(g_sb[:], rec_sb[:], msq_sb[:])
                    for h in range(H):
                        ssl = slice(h * DU, (h + 1) * DU)
                        nc.vector.tensor_mul(
                            v_sb[:, ssl].rearrange("p (d u) -> p d u", u=U),
                            sn_sb[:, ssl].rearrange("p (d u) -> p d u", u=U),
                            g_sb[:, h * U : (h + 1) * U][:, None, :].broadcast_to(
                                [128, D, U]
                            ),
                        )

                    if it == NUM_ITERATIONS - 1:
                        for h in range(H):
                            nc.sync.dma_start(
                                out=out[h * 128 : (h + 1) * 128, :],
                                in_=v_sb[:, h * DU : (h + 1) * DU],
                            )
                    else:
                        # M[k, (d,u)] = sum_b xF[b, k] v[b, (d,u)]; A = Wp .* M
                        for j in range(NT):
                            mp = ps_m.tile([128, DU], f32, tag="m")
                            for h in range(H):
                                nc.tensor.matmul(
                                    mp[:],
                                    xF_sb[
                                        :, h * KL + j * 128 : h * KL + j * 128 + 128
                                    ],
                                    v_sb[:, h * DU : (h + 1) * DU],
                                    start=(h == 0),
                                    stop=(h == H - 1),
                                )
                            nc.vector.tensor_mul(
                                A_sb[:, j * DU : (j + 1) * DU],
                                Wp_sb[:, j * DU : (j + 1) * DU],
                                mp[:],
                            )
                        # reduce over u, then over i (partition groups of 8)
                        nc.vector.tensor_reduce(
                            out=R_sb[:],
                            in_=A_sb[:].rearrange("p (td u) -> p td u", u=U),
                            axis=AX.X,
                            op=ALU.add,
                        )
                        agree_ps = ps_x.tile([128, NT * D], f32, tag="agree")
                        nc.tensor.matmul(
                            agree_ps[:], sel_sb[:], R_sb[:], start=True, stop=True
                        )
                        nc.vector.scalar_tensor_tensor(
                            out=b_sb[:],
                            in0=agree_ps[:],
                            scalar=1.0 / B,
                            in1=b_sb[:],
                            op0=ALU.mult,
                            op1=ALU.add,
                        )

            for _rep in range(repeats):
                emit_pass()

    nc.compile()
    return nc


def prepare_inputs(x, W):
    x = np.ascontiguousarray(np.asarray(x, dtype=np.float32))
    W0 = np.ascontiguousarray(np.asarray(W, dtype=np.float32))[0]  # [ic, nu, us, iu]
    sel = np.kron(np.eye(16, dtype=np.float32), np.ones((8, 8), dtype=np.float32))
    in_maps = []
    for r in range(N_CORES):
        sl = slice(CL * r, CL * (r + 1))
        xl = x[:, :, sl]  # [B, iu, CL]
        xT_r = np.ascontiguousarray(xl.transpose(2, 1, 0).reshape(KL, B))
        xF_r = np.ascontiguousarray(xl.transpose(0, 2, 1).reshape(B, KL))
        Wl = W0[sl]  # [CL, D, U, IU]
        Wp_r = np.ascontiguousarray(Wl.transpose(0, 3, 1, 2).reshape(KL, DU))
        in_maps.append({"xT": xT_r, "xF": xF_r, "Wp": Wp_r, "sel": sel})
    return in_maps


def get_program(repeats=1):
    key = ("nc", repeats)
    if key not in _CACHE:
        _CACHE[key] = build_program(repeats)
    return _CACHE[key]


def run_spmd(in_maps, repeats=1, **kwargs):
    from concourse.bass_utils import run_bass_kernel_spmd

    nc = get_program(repeats)
    return run_bass_kernel_spmd(nc, in_maps, core_ids=list(range(N_CORES)), **kwargs)


def kernel(x, W):
    res = run_spmd(prepare_inputs(x, W))
    v = res.results[0]["out"]  # all cores hold the identical full v
    return np.ascontiguousarray(v.reshape(B, D, U).astype(np.float32))


if __name__ == "__main__":
    xs = np.random.randn(B, IU, IC).astype(np.float32)
    Ws = np.random.randn(1, IC, D, U, IU).astype(np.float32)
    print(kernel(xs, Ws).shape)


# revision 11
# speedup vs baseline: 237.0990x; 237.0990x over previous
"""CapsuleLayer dynamic-routing kernel for 8 Trainium2 NeuronCores.

Sharding: in_channels (ic=1152) split 8 ways (144 per core). Per routing
iteration each core computes its partial s_j over its c-slice; one AllReduce
per iteration sums s-partials (and the softmax denominator, folded into the
same buffer). u_hat is never materialized: both routing contractions are
expressed through the rank structure u_hat = W @ x.

Layouts (per core), with cl = local channel idx (144), i = in_unit (8),
flat k = cl*8 + i (KL = 1152 rows = 9 tiles of 128):
  xT [KL, 256]  : xT[k, b] = x[b, i, c]        (lhsT of the s-matmul)
  xF [256, KL]  : xF[b, k] = x[b, i, c]        (lhsT of the M-matmul)
  Wp [KL, 160]  : Wp[k, d*16+u] = W0[c, d, u, i]
  b_ij state    : b_sb[p, t*10+d] = b[16t + p//8, d]  (replicated over i = p%8)
"""

import sys

if "/opt/trn_rl_repo" not in sys.path:
    sys.path.insert(0, "/opt/trn_rl_repo")

import numpy as np

N_CORES = 8
B, IU, IC, D, U = 256, 8, 1152, 10, 16
CL = IC // N_CORES          # 144 channels per core
KL = CL * IU                # 1152 flat (cl, i) rows per core
NT = KL // 128              # 9 partition tiles
DU = D * U                  # 160
H = B // 128                # 2 batch chunks
NUM_ITERATIONS = 3

_CACHE = {}


def build_program(repeats=1, variant="full"):
    import concourse.mybir as mybir
    import concourse.tile as tile
    from concourse import bacc

    f32 = mybir.dt.float32
    ALU = mybir.AluOpType
    ACTF = mybir.ActivationFunctionType
    AX = mybir.AxisListType

    nc = bacc.Bacc(
        "TRN2",
        target_bir_lowering=False,
        debug=False,
        enable_asserts=False,
        num_devices=N_CORES,
    )

    xT = nc.dram_tensor("xT", [KL, B], f32, kind="ExternalInput")
    xF = nc.dram_tensor("xF", [B, KL], f32, kind="ExternalInput")
    Wp = nc.dram_tensor("Wp", [KL, DU], f32, kind="ExternalInput")
    sel = nc.dram_tensor("sel", [128, 128], f32, kind="ExternalInput")
    out = nc.dram_tensor("out", [B, DU], f32, kind="ExternalOutput")

    with tile.TileContext(nc) as tc:
        with (
            tc.tile_pool(name="big", bufs=1) as bigp,
            tc.tile_pool(name="small", bufs=1) as smp,
            tc.tile_pool(name="ps_s", bufs=2, space="PSUM") as ps_s,
            tc.tile_pool(name="ps_m", bufs=3, space="PSUM") as ps_m,
            tc.tile_pool(name="ps_x", bufs=1, space="PSUM") as ps_x,
            tc.tile_pool(name="dram", bufs=2, space="DRAM") as dpool,
        ):
            # per-k-tile SBUF tiles so DMA -> matmul pipelines at tile grain
            xT_t = [bigp.tile([128, B], f32, tag=f"xT{t}", name=f"xT_sb{t}") for t in range(NT)]
            Wp_t = [bigp.tile([128, DU], f32, tag=f"Wp{t}", name=f"Wp_sb{t}") for t in range(NT)]
            V_t = [bigp.tile([128, DU], f32, tag=f"V{t}", name=f"V_sb{t}") for t in range(NT)]
            A_t = [bigp.tile([128, DU], f32, tag=f"A{t}", name=f"A_sb{t}") for t in range(NT)]
            xF_h = [bigp.tile([128, KL], f32, tag=f"xF{h}", name=f"xF_sb{h}") for h in range(H)]
            sel_sb = smp.tile([128, 128], f32, tag="sel")
            ones_sb = smp.tile([128, 1], f32, tag="ones")
            onesr_sb = smp.tile([1, 128], f32, tag="onesr")
            b_sb = smp.tile([128, NT * D], f32, tag="b")
            cexp_sb = smp.tile([128, NT * D], f32, tag="cexp")
            R_sb = smp.tile([128, NT * D], f32, tag="R")
            s_h = [smp.tile([128, DU], f32, tag=f"s{h}", name=f"s_sb{h}") for h in range(H)]
            spart_sb = smp.tile([128, H * DU], f32, tag="spart")
            sn_sb = smp.tile([128, H * DU], f32, tag="sn")
            sq_sb = smp.tile([128, H * DU], f32, tag="sq")
            v_sb = smp.tile([128, H * DU], f32, tag="v")
            msq_sb = smp.tile([128, H * U], f32, tag="msq")
            sqm_sb = smp.tile([128, H * U], f32, tag="sqm")
            den_sb = smp.tile([128, H * U], f32, tag="den")
            rec_sb = smp.tile([128, H * U], f32, tag="rec")
            g_sb = smp.tile([128, H * U], f32, tag="g")
            zrow_sb = smp.tile([1, DU], f32, tag="zrow")
            ztmp_sb = smp.tile([1, D], f32, tag="ztmp")
            zinv_sb = smp.tile([1, D], f32, tag="zinv")

            def emit_pass():
                # ---- preload (xT/Wp tile-interleaved so s-matmuls start early)
                for t in range(NT):
                    nc.sync.dma_start(
                        out=xT_t[t][:], in_=xT[t * 128 : (t + 1) * 128, :]
                    )
                    nc.sync.dma_start(
                        out=Wp_t[t][:], in_=Wp[t * 128 : (t + 1) * 128, :]
                    )
                for h in range(H):
                    nc.sync.dma_start(
                        out=xF_h[h][:], in_=xF[h * 128 : (h + 1) * 128, :]
                    )
                nc.sync.dma_start(out=sel_sb[:], in_=sel[:])
                nc.gpsimd.memset(ones_sb[:], 1.0)
                nc.gpsimd.memset(onesr_sb[:], 1.0)
                nc.gpsimd.memset(b_sb[:], 0.0)
                nc.gpsimd.memset(zrow_sb[:], 0.0)
                # iteration 0: c = exp(0) = 1 everywhere; local Z8 = 8*144
                nc.gpsimd.memset(zrow_sb[0:1, 0:D], float(IC))

                for it in range(NUM_ITERATIONS):
                    if it == 0:
                        Vcur = Wp_t
                    else:
                        # c-scale: cexp = exp(b); Z8_local[d] = sum_{p,t} cexp
                        nc.scalar.activation(cexp_sb[:], b_sb[:], ACTF.Exp)
                        zsum_ps = ps_x.tile([1, NT * D], f32, tag="zsum")
                        nc.tensor.matmul(
                            zsum_ps[:], ones_sb[:], cexp_sb[:], start=True, stop=True
                        )
                        nc.vector.tensor_reduce(
                            out=zrow_sb[0:1, 0:D],
                            in_=zsum_ps[0:1, :].rearrange("p (t d) -> p d t", d=D),
                            axis=AX.X,
                            op=ALU.add,
                        )
                        for t in range(NT):
                            nc.vector.tensor_mul(
                                V_t[t][:].rearrange("p (d u) -> p d u", u=U),
                                Wp_t[t][:].rearrange("p (d u) -> p d u", u=U),
                                cexp_sb[:, t * D : (t + 1) * D][:, :, None]
                                .broadcast_to([128, D, U]),
                            )
                        Vcur = V_t

                    # s-matmul: s_unnorm[b, (d,u)] = sum_k xT[k, b] * V[k, (d,u)]
                    s_tiles = []
                    for h in range(H):
                        sp = ps_s.tile([128, DU], f32, tag="s")
                        for t in range(NT):
                            nc.tensor.matmul(
                                sp[:],
                                xT_t[t][:, h * 128 : h * 128 + 128],
                                Vcur[t][:],
                                start=(t == 0),
                                stop=(t == NT - 1),
                            )
                        s_tiles.append(sp)

                    # AllReduce of [s_partial (256 rows) ; Z8 row]
                    cc_in = dpool.tile([B + 1, DU], f32, tag="ccin")
                    cc_out = dpool.tile([B + 1, DU], f32, tag="ccout")
                    for h in range(H):
                        nc.scalar.copy(
                            spart_sb[:, h * DU : (h + 1) * DU], s_tiles[h][:]
                        )
                        nc.sync.dma_start(
                            out=cc_in[h * 128 : (h + 1) * 128, :],
                            in_=spart_sb[:, h * DU : (h + 1) * DU],
                        )
                    nc.sync.dma_start(out=cc_in[B : B + 1, :], in_=zrow_sb[:])
                    if variant == "nocc":
                        # timing-ablation only: skip the cross-core reduce
                        nc.sync.dma_start(out=cc_out.opt(), in_=cc_in.opt())
                    else:
                        nc.gpsimd.collective_compute(
                            "AllReduce",
                            ALU.add,
                            replica_groups=[list(range(N_CORES))],
                            ins=[cc_in.opt()],
                            outs=[cc_out.opt()],
                        )
                    # z first: the zinv chain runs while s chunks stream back
                    nc.sync.dma_start(out=zrow_sb[:], in_=cc_out[B : B + 1, :])
                    for h in range(H):
                        nc.sync.dma_start(
                            out=s_h[h][:], in_=cc_out[h * 128 : (h + 1) * 128, :]
                        )

                    # zinv[d] = 1 / (Z8_AR[d] / 8); broadcast across partitions
                    nc.scalar.mul(ztmp_sb[:], zrow_sb[0:1, 0:D], 1.0 / N_CORES)
                    nc.vector.reciprocal(zinv_sb[:], ztmp_sb[:])
                    zb_ps = ps_x.tile([128, D], f32, tag="zb")
                    nc.tensor.matmul(
                        zb_ps[:], onesr_sb[:], zinv_sb[:], start=True, stop=True
                    )

                    # squash (norm over d per (b, u), faithful to reference)
                    for h in range(H):
                        ssl = slice(h * DU, (h + 1) * DU)
                        nc.vector.tensor_mul(
                            sn_sb[:, ssl].rearrange("p (d u) -> p d u", u=U),
                            s_h[h][:].rearrange("p (d u) -> p d u", u=U),
                            zb_ps[:, :, None].broadcast_to([128, D, U]),
                        )
                        nc.scalar.square(sq_sb[:, ssl], sn_sb[:, ssl])
                        nc.vector.tensor_reduce(
                            out=msq_sb[:, h * U : (h + 1) * U],
                            in_=sq_sb[:, ssl].rearrange("p (d u) -> p u d", u=U),
                            axis=AX.X,
                            op=ALU.add,
                        )
                    nc.scalar.sqrt(sqm_sb[:], msq_sb[:])
                    nc.vector.scalar_tensor_tensor(
                        out=den_sb[:],
                        in0=msq_sb[:],
                        scalar=1.0,
                        in1=sqm_sb[:],
                        op0=ALU.add,
                        op1=ALU.mult,
                    )
                    nc.vector.reciprocal(rec_sb[:], den_sb[:])
                    nc.vector.tensor_mul(g_sb[:], rec_sb[:], msq_sb[:])
                    for h in range(H):
                        ssl = slice(h * DU, (h + 1) * DU)
                        nc.vector.tensor_mul(
                            v_sb[:, ssl].rearrange("p (d u) -> p d u", u=U),
                            sn_sb[:, ssl].rearrange("p (d u) -> p d u", u=U),
                            g_sb[:, h * U : (h + 1) * U][:, None, :].broadcast_to(
                                [128, D, U]
                            ),
                        )

                    if it == NUM_ITERATIONS - 1:
                        for h in range(H):
                            nc.sync.dma_start(
                                out=out[h * 128 : (h + 1) * 128, :],
                                in_=v_sb[:, h * DU : (h + 1) * DU],
                            )
                    else:
                        # M[k, (d,u)] = sum_b xF[b, k] v[b, (d,u)]; A = Wp .* M
                        for j in range(NT):
                            mp = ps_m.tile([128, DU], f32, tag="m")
                            for h in range(H):
                                nc.tensor.matmul(
                                    mp[:],
                                    xF_h[h][:, j * 128 : j * 128 + 128],
                                    v_sb[:, h * DU : (h + 1) * DU],
                                    start=(h == 0),
                                    stop=(h == H - 1),
                                )
                            nc.vector.tensor_mul(A_t[j][:], Wp_t[j][:], mp[:])
                            # reduce over u, pipelined per tile
                            nc.vector.tensor_reduce(
                                out=R_sb[:, j * D : (j + 1) * D],
                                in_=A_t[j][:].rearrange("p (d u) -> p d u", u=U),
                                axis=AX.X,
                                op=ALU.add,
                            )
                        # reduce over i (partition groups of 8) via PE
                        agree_ps = ps_x.tile([128, NT * D], f32, tag="agree")
                        nc.tensor.matmul(
                            agree_ps[:], sel_sb[:], R_sb[:], start=True, stop=True
                        )
                        nc.vector.scalar_tensor_tensor(
                            out=b_sb[:],
                            in0=agree_ps[:],
                            scalar=1.0 / B,
                            in1=b_sb[:],
                            op0=ALU.mult,
                            op1=ALU.add,
                        )

            for _rep in range(repeats):
                emit_pass()

    nc.compile()
    return nc


def prepare_inputs(x, W):
    x = np.ascontiguousarray(np.asarray(x, dtype=np.float32))
    W0 = np.ascontiguousarray(np.asarray(W, dtype=np.float32))[0]  # [ic, nu, us, iu]
    sel = np.kron(np.eye(16, dtype=np.float32), np.ones((8, 8), dtype=np.float32))
    in_maps = []
    for r in range(N_CORES):
        sl = slice(CL * r, CL * (r + 1))
        xl = x[:, :, sl]  # [B, iu, CL]
        xT_r = np.ascontiguousarray(xl.transpose(2, 1, 0).reshape(KL, B))
        xF_r = np.ascontiguousarray(xl.transpose(0, 2, 1).reshape(B, KL))
        Wl = W0[sl]  # [CL, D, U, IU]
        Wp_r = np.ascontiguousarray(Wl.transpose(0, 3, 1, 2).reshape(KL, DU))
        in_maps.append({"xT": xT_r, "xF": xF_r, "Wp": Wp_r, "sel": sel})
    return in_maps


def get_program(repeats=1, variant="full"):
    key = ("nc", repeats, variant)
    if key not in _CACHE:
        _CACHE[key] = build_program(repeats, variant)
    return _CACHE[key]


def run_spmd(in_maps, repeats=1, variant="full", **kwargs):
    from concourse.bass_utils import run_bass_kernel_spmd

    nc = get_program(repeats, variant)
    return run_bass_kernel_spmd(nc, in_maps, core_ids=list(range(N_CORES)), **kwargs)


def kernel(x, W):
    res = run_spmd(prepare_inputs(x, W))
    v = res.results[0]["out"]  # all cores hold the identical full v
    return np.ascontiguousarray(v.reshape(B, D, U).astype(np.float32))


if __name__ == "__main__":
    xs = np.random.randn(B, IU, IC).astype(np.float32)
    Ws = np.random.randn(1, IC, D, U, IU).astype(np.float32)
    print(kernel(xs, Ws).shape)


# revision 15
# speedup vs baseline: 250.9741x; 1.0585x over previous
"""CapsuleLayer dynamic-routing kernel for 8 Trainium2 NeuronCores.

Sharding: in_channels (ic=1152) split 8 ways (144 per core). Per routing
iteration each core computes its partial s_j over its c-slice; one AllReduce
per iteration sums s-partials (and the softmax denominator, folded into the
same buffer). u_hat is never materialized: both routing contractions are
expressed through the rank structure u_hat = W @ x.

Layouts (per core), with cl = local channel idx (144), i = in_unit (8),
flat k = cl*8 + i (KL = 1152 rows = 9 tiles of 128):
  xT [KL, 256]  : xT[k, b] = x[b, i, c]        (lhsT of the s-matmul)
  xF [256, KL]  : xF[b, k] = x[b, i, c]        (lhsT of the M-matmul)
  Wp [KL, 160]  : Wp[k, d*16+u] = W0[c, d, u, i]
  b_ij state    : b_sb[p, t*10+d] = b[16t + p//8, d]  (replicated over i = p%8)
"""

import sys

if "/opt/trn_rl_repo" not in sys.path:
    sys.path.insert(0, "/opt/trn_rl_repo")

import numpy as np

N_CORES = 8
B, IU, IC, D, U = 256, 8, 1152, 10, 16
CL = IC // N_CORES          # 144 channels per core
KL = CL * IU                # 1152 flat (cl, i) rows per core
NT = KL // 128              # 9 partition tiles
DU = D * U                  # 160
H = B // 128                # 2 batch chunks
NUM_ITERATIONS = 3

_CACHE = {}


def build_program(repeats=1, variant="full"):
    import concourse.mybir as mybir
    import concourse.tile as tile
    from concourse import bacc

    f32 = mybir.dt.float32
    ALU = mybir.AluOpType
    ACTF = mybir.ActivationFunctionType
    AX = mybir.AxisListType

    nc = bacc.Bacc(
        "TRN2",
        target_bir_lowering=False,
        debug=False,
        enable_asserts=False,
        num_devices=N_CORES,
    )

    xT = nc.dram_tensor("xT", [KL, B], f32, kind="ExternalInput")
    xF = nc.dram_tensor("xF", [B, KL], f32, kind="ExternalInput")
    Wp = nc.dram_tensor("Wp", [KL, DU], f32, kind="ExternalInput")
    sel = nc.dram_tensor("sel", [128, 128], f32, kind="ExternalInput")
    out = nc.dram_tensor("out", [B, DU], f32, kind="ExternalOutput")

    with tile.TileContext(nc) as tc:
        with (
            tc.tile_pool(name="big", bufs=1) as bigp,
            tc.tile_pool(name="small", bufs=1) as smp,
            tc.tile_pool(name="ps_s", bufs=2, space="PSUM") as ps_s,
            tc.tile_pool(name="ps_m", bufs=3, space="PSUM") as ps_m,
            tc.tile_pool(name="ps_x", bufs=1, space="PSUM") as ps_x,
            tc.tile_pool(name="dram", bufs=2, space="DRAM") as dpool,
        ):
            # per-k-tile SBUF tiles so DMA -> matmul pipelines at tile grain
            xT_t = [bigp.tile([128, B], f32, tag=f"xT{t}", name=f"xT_sb{t}") for t in range(NT)]
            Wp_t = [bigp.tile([128, DU], f32, tag=f"Wp{t}", name=f"Wp_sb{t}") for t in range(NT)]
            V_t = [bigp.tile([128, DU], f32, tag=f"V{t}", name=f"V_sb{t}") for t in range(NT)]
            A_t = [bigp.tile([128, DU], f32, tag=f"A{t}", name=f"A_sb{t}") for t in range(NT)]
            xF_h = [bigp.tile([128, KL], f32, tag=f"xF{h}", name=f"xF_sb{h}") for h in range(H)]
            sel_sb = smp.tile([128, 128], f32, tag="sel")
            ones_sb = smp.tile([128, 1], f32, tag="ones")
            onesr_sb = smp.tile([1, 128], f32, tag="onesr")
            b_sb = smp.tile([128, NT * D], f32, tag="b")
            cexp_sb = smp.tile([128, NT * D], f32, tag="cexp")
            R_sb = smp.tile([128, NT * D], f32, tag="R")
            s_h = [smp.tile([128, DU], f32, tag=f"s{h}", name=f"s_sb{h}") for h in range(H)]
            spart_sb = smp.tile([128, H * DU], f32, tag="spart")
            sn_sb = smp.tile([128, H * DU], f32, tag="sn")
            sq_sb = smp.tile([128, H * DU], f32, tag="sq")
            v_sb = smp.tile([128, H * DU], f32, tag="v")
            msq_sb = smp.tile([128, H * U], f32, tag="msq")
            sqm_sb = smp.tile([128, H * U], f32, tag="sqm")
            den_sb = smp.tile([128, H * U], f32, tag="den")
            rec_sb = smp.tile([128, H * U], f32, tag="rec")
            g_sb = smp.tile([128, H * U], f32, tag="g")
            zrow_sb = smp.tile([1, DU], f32, tag="zrow")
            ztmp_sb = smp.tile([1, D], f32, tag="ztmp")
            zinv_sb = smp.tile([1, D], f32, tag="zinv")

            def emit_pass():
                # ---- preload (xT/Wp tile-interleaved so s-matmuls start early)
                for t in range(NT):
                    nc.sync.dma_start(
                        out=xT_t[t][:], in_=xT[t * 128 : (t + 1) * 128, :]
                    )
                    nc.sync.dma_start(
                        out=Wp_t[t][:], in_=Wp[t * 128 : (t + 1) * 128, :]
                    )
                for h in range(H):
                    for q in range(4):
                        nc.sync.dma_start(
                            out=xF_h[h][32 * q : 32 * q + 32, :],
                            in_=xF[h * 128 + 32 * q : h * 128 + 32 * q + 32, :],
                        )
                nc.sync.dma_start(out=sel_sb[:], in_=sel[:])
                nc.gpsimd.memset(ones_sb[:], 1.0)
                nc.gpsimd.memset(onesr_sb[:], 1.0)
                nc.gpsimd.memset(b_sb[:], 0.0)
                nc.gpsimd.memset(zrow_sb[:], 0.0)
                # iteration 0: c = exp(0) = 1 everywhere; local Z8 = 8*144
                nc.gpsimd.memset(zrow_sb[0:1, 0:D], float(IC))

                for it in range(NUM_ITERATIONS):
                    if it == 0:
                        Vcur = Wp_t
                    else:
                        # c-scale: cexp = exp(b); Z8_local[d] = sum_{p,t} cexp
                        nc.scalar.activation(cexp_sb[:], b_sb[:], ACTF.Exp)
                        zsum_ps = ps_x.tile([1, NT * D], f32, tag="zsum")
                        nc.tensor.matmul(
                            zsum_ps[:], ones_sb[:], cexp_sb[:], start=True, stop=True
                        )
                        nc.vector.tensor_reduce(
                            out=zrow_sb[0:1, 0:D],
                            in_=zsum_ps[0:1, :].rearrange("p (t d) -> p d t", d=D),
                            axis=AX.X,
                            op=ALU.add,
                        )
                        for t in range(NT):
                            nc.vector.tensor_mul(
                                V_t[t][:].rearrange("p (d u) -> p d u", u=U),
                                Wp_t[t][:].rearrange("p (d u) -> p d u", u=U),
                                cexp_sb[:, t * D : (t + 1) * D][:, :, None]
                                .broadcast_to([128, D, U]),
                            )
                        Vcur = V_t

                    # s-matmul: s_unnorm[b, (d,u)] = sum_k xT[k, b] * V[k, (d,u)]
                    s_tiles = []
                    for h in range(H):
                        sp = ps_s.tile([128, DU], f32, tag="s")
                        for t in range(NT):
                            nc.tensor.matmul(
                                sp[:],
                                xT_t[t][:, h * 128 : h * 128 + 128],
                                Vcur[t][:],
                                start=(t == 0),
                                stop=(t == NT - 1),
                            )
                        s_tiles.append(sp)

                    # AllReduce of [s_partial (256 rows) ; Z8 row]
                    cc_in = dpool.tile([B + 1, DU], f32, tag="ccin")
                    cc_out = dpool.tile(
                        [B + 1, DU],
                        f32,
                        tag="ccout",
                        addr_space="Shared" if variant == "shared" else "Local",
                    )
                    for h in range(H):
                        nc.scalar.copy(
                            spart_sb[:, h * DU : (h + 1) * DU], s_tiles[h][:]
                        )
                        # split across DMA queues: one dma_start is one queue
                        for q in range(4):
                            nc.sync.dma_start(
                                out=cc_in[h * 128 + 32 * q : h * 128 + 32 * q + 32, :],
                                in_=spart_sb[
                                    32 * q : 32 * q + 32, h * DU : (h + 1) * DU
                                ],
                            )
                    nc.sync.dma_start(out=cc_in[B : B + 1, :], in_=zrow_sb[:])
                    if variant == "nocc":
                        # timing-ablation only: skip the cross-core reduce
                        nc.sync.dma_start(out=cc_out.opt(), in_=cc_in.opt())
                    else:
                        nc.gpsimd.collective_compute(
                            "AllReduce",
                            ALU.add,
                            replica_groups=[list(range(N_CORES))],
                            ins=[cc_in.opt()],
                            outs=[cc_out.opt()],
                        )
                    # z first: the zinv chain runs while s chunks stream back
                    nc.sync.dma_start(out=zrow_sb[:], in_=cc_out[B : B + 1, :])
                    for h in range(H):
                        for q in range(4):
                            nc.sync.dma_start(
                                out=s_h[h][32 * q : 32 * q + 32, :],
                                in_=cc_out[
                                    h * 128 + 32 * q : h * 128 + 32 * q + 32, :
                                ],
                            )

                    # zinv[d] = 1 / (Z8_AR[d] / 8); broadcast across partitions
                    nc.scalar.mul(ztmp_sb[:], zrow_sb[0:1, 0:D], 1.0 / N_CORES)
                    nc.vector.reciprocal(zinv_sb[:], ztmp_sb[:])
                    zb_ps = ps_x.tile([128, D], f32, tag="zb")
                    nc.tensor.matmul(
                        zb_ps[:], onesr_sb[:], zinv_sb[:], start=True, stop=True
                    )

                    # squash (norm over d per (b, u), faithful to reference)
                    for h in range(H):
                        ssl = slice(h * DU, (h + 1) * DU)
                        nc.vector.tensor_mul(
                            sn_sb[:, ssl].rearrange("p (d u) -> p d u", u=U),
                            s_h[h][:].rearrange("p (d u) -> p d u", u=U),
                            zb_ps[:, :, None].broadcast_to([128, D, U]),
                        )
                        nc.scalar.square(sq_sb[:, ssl], sn_sb[:, ssl])
                        nc.vector.tensor_reduce(
                            out=msq_sb[:, h * U : (h + 1) * U],
                            in_=sq_sb[:, ssl].rearrange("p (d u) -> p u d", u=U),
                            axis=AX.X,
                            op=ALU.add,
                        )
                    nc.scalar.sqrt(sqm_sb[:], msq_sb[:])
                    nc.vector.scalar_tensor_tensor(
                        out=den_sb[:],
                        in0=msq_sb[:],
                        scalar=1.0,
                        in1=sqm_sb[:],
                        op0=ALU.add,
                        op1=ALU.mult,
                    )
                    nc.vector.reciprocal(rec_sb[:], den_sb[:])
                    nc.vector.tensor_mul(g_sb[:], rec_sb[:], msq_sb[:])
                    for h in range(H):
                        ssl = slice(h * DU, (h + 1) * DU)
                        nc.vector.tensor_mul(
                            v_sb[:, ssl].rearrange("p (d u) -> p d u", u=U),
                            sn_sb[:, ssl].rearrange("p (d u) -> p d u", u=U),
                            g_sb[:, h * U : (h + 1) * U][:, None, :].broadcast_to(
                                [128, D, U]
                            ),
                        )

                    if it == NUM_ITERATIONS - 1:
                        for h in range(H):
                            nc.sync.dma_start(
                                out=out[h * 128 : (h + 1) * 128, :],
                                in_=v_sb[:, h * DU : (h + 1) * DU],
                            )
                    else:
                        # M[k, (d,u)] = sum_b xF[b, k] v[b, (d,u)]; A = Wp .* M
                        for j in range(NT):
                            mp = ps_m.tile([128, DU], f32, tag="m")
                            for h in range(H):
                                nc.tensor.matmul(
                                    mp[:],
                                    xF_h[h][:, j * 128 : j * 128 + 128],
                                    v_sb[:, h * DU : (h + 1) * DU],
                                    start=(h == 0),
                                    stop=(h == H - 1),
                                )
                            nc.vector.tensor_mul(A_t[j][:], Wp_t[j][:], mp[:])
                            # reduce over u, pipelined per tile
                            nc.vector.tensor_reduce(
                                out=R_sb[:, j * D : (j + 1) * D],
                                in_=A_t[j][:].rearrange("p (d u) -> p d u", u=U),
                                axis=AX.X,
                                op=ALU.add,
                            )
                        # reduce over i (partition groups of 8) via PE
                        agree_ps = ps_x.tile([128, NT * D], f32, tag="agree")
                        nc.tensor.matmul(
                            agree_ps[:], sel_sb[:], R_sb[:], start=True, stop=True
                        )
                        nc.vector.scalar_tensor_tensor(
                            out=b_sb[:],
                            in0=agree_ps[:],
                            scalar=1.0 / B,
                            in1=b_sb[:],
                            op0=ALU.mult,
                            op1=ALU.add,
                        )

            for _rep in range(repeats):
                emit_pass()

    nc.compile()
    return nc


def prepare_inputs(x, W):
    x = np.ascontiguousarray(np.asarray(x, dtype=np.float32))
    W0 = np.ascontiguousarray(np.asarray(W, dtype=np.float32))[0]  # [ic, nu, us, iu]
    sel = np.kron(np.eye(16, dtype=np.float32), np.ones((8, 8), dtype=np.float32))
    in_maps = []
    for r in range(N_CORES):
        sl = slice(CL * r, CL * (r + 1))
        xl = x[:, :, sl]  # [B, iu, CL]
        xT_r = np.ascontiguousarray(xl.transpose(2, 1, 0).reshape(KL, B))
        xF_r = np.ascontiguousarray(xl.transpose(0, 2, 1).reshape(B, KL))
        Wl = W0[sl]  # [CL, D, U, IU]
        Wp_r = np.ascontiguousarray(Wl.transpose(0, 3, 1, 2).reshape(KL, DU))
        in_maps.append({"xT": xT_r, "xF": xF_r, "Wp": Wp_r, "sel": sel})
    return in_maps


def get_program(repeats=1, variant="full"):
    key = ("nc", repeats, variant)
    if key not in _CACHE:
        _CACHE[key] = build_program(repeats, variant)
    return _CACHE[key]


def run_spmd(in_maps, repeats=1, variant="full", **kwargs):
    from concourse.bass_utils import run_bass_kernel_spmd

    nc = get_program(repeats, variant)
    return run_bass_kernel_spmd(nc, in_maps, core_ids=list(range(N_CORES)), **kwargs)


def kernel(x, W):
    res = run_spmd(prepare_inputs(x, W))
    v = res.results[0]["out"]  # all cores hold the identical full v
    return np.ascontiguousarray(v.reshape(B, D, U).astype(np.float32))


if __name__ == "__main__":
    xs = np.random.randn(B, IU, IC).astype(np.float32)
    Ws = np.random.randn(1, IC, D, U, IU).astype(np.float32)
    print(kernel(xs, Ws).shape)
